# revision 24
# baseline (speedup 1.0000x reference)
"""Multi-head attention (B=2, S=2048, D=1024, H=16) on 8 TRN2 NeuronCores.

Sharding: core c handles batch b = c//4 and heads [4*(c%4), 4*(c%4)+4) —
tensor-parallel over heads x data-parallel over batch.  Each core computes a
partial output projection (its heads' contribution); the host sums the 4
partials per batch and adds b_out.

Schedule (per core): the kernel is paced so the PE never idles long enough
for the HAM clock-gate to re-throttle it to 1.2 GHz:
  - 16 "steps" per attention block, one sk-chunk each: 2 concurrent score
    matmuls (row-bands 0-63/64-127 for the head pair) into one 2-bank PSUM
    tile, then ~1 filler matmul group, then one exp covering both heads.
  - filler groups are the previous block's attn@v chains (split into 4-matmul
    groups), the v/qk projections (first block), and the output projection
    (later blocks), so PE work is dense through the ACT-paced score phase.
  - a tunable subset of exp steps runs on the DVE as a Schraudolph bit-trick
    (u16 = round(a*score + b) bitcast bf16) so the ACT engine stays strictly
    ahead of the PE.
  - softmax denominators (ones-column rows from attn@v) are transposed into
    partition-columns by DMA, inverted in one cheap batched DVE reciprocal,
    bounced through DRAM for the partition-broadcast, and multiplied into
    the bf16 vT staging the output projection consumes.
"""
import sys

sys.path.insert(0, "/opt/trn_rl_repo")

import numpy as np

B, S, D = 2, 2048, 1024
H, Hd = 16, 64
P = 128
NKC = D // P      # 8 contraction chunks for the projections
NSC = S // P      # 16 sequence chunks of 128
SQB = 512         # sq block size
NSQB = S // SQB   # 4

# Schraudolph exp on DVE: bf16 bits = EXP_A * score + EXP_B
EXP_A = 184.66246 * 0.125
EXP_B = 16249.0

# blocks processed in (sqb-major, p-interleaved) order
BLOCKS = [(0, 0), (1, 0), (0, 1), (1, 1), (0, 2), (1, 2), (0, 3), (1, 3)]
# steps whose exp runs on DVE (Schraudolph) instead of ACT, per block index
SCHR = {
    1: (1, 3, 5, 7, 9, 11, 13),
    2: (1, 3, 5, 7, 9, 11, 13),
    3: (3, 5, 7, 9, 11, 13),
    4: (3, 5, 7, 9, 11, 13),
    5: (3, 5, 7, 9, 11, 13),
    6: (3, 5, 7, 9, 11, 13),
    7: (3, 5, 7, 9, 11, 13),
}
# step placement of the previous block's attn@v 4-matmul groups, (g, sub)
# pairs in order: chains end by step 9 so the denominator/value evacuation
# and the normalize chain complete well before the next block consumes them
AV_STEPS = (1, 2, 3, 4, 5, 6, 7, 9)
# out-proj (mm_step, cast_step) placement: first half of a sqb's tiles in
# the late steps of one block, second half early in the next (vT for the
# second head pair lands ~step 6); None = after the step loop
OP_A = ((11, 13), (12, 14), (13, 15), (14, None))
OP_B = ((0, 2), (1, 3), (2, 4), (3, 5))
OP_FULL = ((8, 10), (9, 11), (10, 12), (11, 13), (12, 14), (13, 15),
           (14, None), (15, None))

_CACHE = {}


def _build_nc():
    import concourse.mybir as mybir
    import concourse.tile as tile
    from concourse import bacc

    f32 = mybir.dt.float32
    bf16 = mybir.dt.bfloat16
    i16 = mybir.dt.int16
    AF = mybir.ActivationFunctionType
    ALU = mybir.AluOpType

    nc = bacc.Bacc(None, target_bir_lowering=False, debug=False)

    yT_d = nc.dram_tensor("yT", [D, S], bf16, kind="ExternalInput")[:]
    Wqk_d = nc.dram_tensor("WqkT", [D, 512], bf16, kind="ExternalInput")[:]
    bqk_d = nc.dram_tensor("bqk", [P, 4], f32, kind="ExternalInput")[:]
    Wv_d = nc.dram_tensor("WvT", [D, 256], bf16, kind="ExternalInput")[:]
    Wout_d = nc.dram_tensor("WoutT", [256, D], bf16, kind="ExternalInput")[:]
    out_d = nc.dram_tensor("out", [S, D], bf16, kind="ExternalOutput")[:]

    with tile.TileContext(nc) as tc:
        with (
            tc.tile_pool(name="const", bufs=1) as const,
            tc.tile_pool(name="persist", bufs=1) as persist,
        ):
            qkT_sb = persist.tile([P, 4, S], bf16)
            v_sb = persist.tile([P, NSC, 4, 65], bf16)
            vT_sb = persist.tile([P, 2, S], bf16)
            den_cols = persist.tile([P, 64], f32)
            rec_cols = persist.tile([P, 64], f32)
            Wout_sb = const.tile([P, 2, D], bf16)
            bqk_sb = const.tile([P, 4], f32)
            ones_f32 = const.tile([P, 1], f32)
            ones_row = const.tile([P, 64], f32)

            p1 = tc.alloc_tile_pool(name="p1", bufs=1)
            Wqk_sb = p1.tile([P, NKC, 512], bf16)
            Wv_sb = p1.tile([P, NKC, 256], bf16)
            yT_sb = p1.tile([P, NKC, S], bf16)

            # input DMAs: per-chunk, alternating the two HWDGE queues, in
            # first-use order (Wqk + the sb=0 quarter of yT feed the first
            # projection chains) so the PE starts after ~2 MB, not 6.
            Wqkr = Wqk_d.rearrange("(kc p) e -> p kc e", p=P)
            yTr = yT_d.rearrange("(kc p) s -> p kc s", p=P)
            Wvr = Wv_d.rearrange("(kc p) e -> p kc e", p=P)
            nc.sync.dma_start(bqk_sb[:], bqk_d)
            for kc in range(NKC):
                nc.sync.dma_start(Wqk_sb[:, kc], Wqkr[:, kc])
            for kp in range(4):
                nc.scalar.dma_start(yT_sb[:, 2 * kp:2 * kp + 2, 0:512],
                                    yTr[:, 2 * kp:2 * kp + 2, 0:512])
            for sb in range(1, 4):
                for kp in range(4):
                    eng = nc.gpsimd if sb >= 2 else nc.scalar
                    eng.dma_start(
                        yT_sb[:, 2 * kp:2 * kp + 2,
                              sb * 512:(sb + 1) * 512],
                        yTr[:, 2 * kp:2 * kp + 2,
                            sb * 512:(sb + 1) * 512])
            for h in range(2):
                nc.gpsimd.dma_start(Wv_sb[:, 4 * h:4 * h + 4],
                                    Wvr[:, 4 * h:4 * h + 4])
            nc.sync.dma_start(
                Wout_sb[:], Wout_d.rearrange("(kc p) e -> p kc e", p=P))
            nc.any.memset(ones_f32[:], 1.0)
            nc.any.memset(ones_row[:], 1.0)
            nc.vector.tensor_copy(
                v_sb[:, :, :, 64:65],
                ones_f32.unsqueeze(1).unsqueeze(1).to_broadcast(
                    (P, NSC, 4, 1)))

            with (
                tc.tile_pool(name="p2e", bufs=1) as p2e,
                tc.tile_pool(name="p2s", bufs=1) as p2s,
                tc.tile_pool(name="p2ps", bufs=1, space="PSUM") as p2ps,
                tc.tile_pool(name="p2dram", bufs=2, space="DRAM") as p2dram,
            ):
                # ---- matmul unit generators (each ~one 4-8 matmul group,
                # issued between score steps as PE filler) ----
                def proj_unit(m, sb):
                    def f():
                        ps = p2ps.tile([P, 512], f32, tag="pjav", bufs=2,
                                       name="ps")
                        for kc in range(NKC):
                            nc.tensor.matmul(
                                ps[:],
                                Wqk_sb[:, kc, m * P:(m + 1) * P],
                                yT_sb[:, kc, sb * 512:(sb + 1) * 512],
                                start=(kc == 0), stop=(kc == NKC - 1))
                        nc.scalar.activation(
                            qkT_sb[:, m, sb * 512:(sb + 1) * 512], ps[:],
                            AF.Identity, bias=bqk_sb[:, m:m + 1])
                    return f

                def vproj_unit(sc):
                    def f():
                        psv = p2ps.tile([P, 256], f32, tag="aux", bufs=2,
                                        name="psv")
                        for kc in range(NKC):
                            nc.tensor.matmul(
                                psv[:],
                                yT_sb[:, kc, sc * P:(sc + 1) * P],
                                Wv_sb[:, kc, :],
                                start=(kc == 0), stop=(kc == NKC - 1))
                        nc.vector.tensor_copy(
                            v_sb[:, sc, :, 0:64],
                            psv.rearrange("p (i d) -> p i d", i=4))
                    return f

                av_tiles = {}
                nrm = {}
                last_ex = [None]

                def attnv_unit(p_, sqb_, sub, g, ex_t):
                    def f():
                        ex = ex_t if ex_t is not None else last_ex[0]
                        key = (p_, sqb_, sub)
                        if g == 0:
                            av_tiles[key] = p2ps.tile(
                                [P, SQB], f32, tag="pjav", bufs=2,
                                name="psv2")
                        psv2 = av_tiles[key]
                        i = 2 * p_ + sub
                        for mk in range(4 * g, 4 * g + 4):
                            nc.tensor.matmul(
                                psv2[0:65, :],
                                v_sb[:, mk, i, :],
                                ex[:, mk, sub, :],
                                start=(mk == 0), stop=(mk == NSC - 1))
                    return f

                def av_copies(p_, sqb_, sub):
                    """Evacuate one finished attn@v PSUM tile: den row and
                    unnormalized values to SBUF, split across ACT/DVE."""
                    def f():
                        psv2 = av_tiles.pop((p_, sqb_, sub))
                        if sub == 0:
                            nrm[(p_, sqb_, "den")] = p2s.tile(
                                [P, 2, SQB], f32, tag="den", bufs=2,
                                name="den_sb")
                        den_sb = nrm[(p_, sqb_, "den")]
                        vals = p2s.tile([64, SQB], bf16, tag="vals",
                                        bufs=4, name="vals")
                        if sub == 0:
                            nc.scalar.copy(den_sb[64:65, 0, :],
                                           psv2[64:65, :])
                            nc.vector.tensor_copy(vals[:], psv2[0:64, :])
                        else:
                            nc.vector.tensor_copy(den_sb[64:65, 1, :],
                                                  psv2[64:65, :])
                            nc.scalar.copy(vals[:], psv2[0:64, :])
                        nrm[(p_, sqb_, sub)] = vals
                    return f

                def norm_a(bi_prev):
                    """Denominator rows -> DRAM -> transposed columns
                    (DMA only, no engine time)."""
                    def f():
                        p_, sqb_ = BLOCKS[bi_prev]
                        base = bi_prev * 8
                        den_sb = nrm[(p_, sqb_, "den")]
                        den_dram = p2dram.tile([2, SQB], f32,
                                               name="den_dram", bufs=2)
                        for s in (0, 1):
                            nc.gpsimd.dma_start(den_dram[s:s + 1, :],
                                                den_sb[64:65, s, :])
                        nc.gpsimd.dma_start(
                            den_cols[:, base:base + 8].rearrange(
                                "p (s j) -> p s j", s=2),
                            den_dram.rearrange("s (j p) -> p s j", p=P))
                    return f

                def norm_b1(bi_prev):
                    """Batched reciprocal of the transposed denominators,
                    then DRAM bounce for the partition broadcast."""
                    def f():
                        p_, sqb_ = BLOCKS[bi_prev]
                        base = bi_prev * 8
                        nrm.pop((p_, sqb_, "den"))
                        nc.vector.reciprocal(rec_cols[:, base:base + 8],
                                             den_cols[:, base:base + 8])
                        rdram = p2dram.tile([2, SQB], f32, name="rdram",
                                            bufs=2)
                        nc.sync.dma_start(
                            rdram.rearrange("s (j p) -> p s j", p=P),
                            rec_cols[:, base:base + 8].rearrange(
                                "p (s j) -> p s j", s=2))
                        rbs = p2s.tile([64, 2, SQB], f32, tag="rbs",
                                       bufs=2, name="rbs")
                        for s in (0, 1):
                            nc.sync.dma_start(
                                rbs[:, s, :],
                                rdram[s:s + 1, :].to_broadcast((64, SQB)))
                        nrm[(p_, sqb_, "rbs")] = rbs
                    return f

                def norm_b2(bi_prev, sub):
                    """Normalize one head's values into vT_sb."""
                    def f():
                        p_, sqb_ = BLOCKS[bi_prev]
                        sq = slice(sqb_ * SQB, (sqb_ + 1) * SQB)
                        rbs = nrm[(p_, sqb_, "rbs")]
                        vals = nrm.pop((p_, sqb_, sub))
                        if sub == 0:
                            nc.gpsimd.tensor_mul(vT_sb[0:64, p_, sq],
                                                 vals[:], rbs[:, 0, :])
                        else:
                            vstage = p2s.tile([64, SQB], bf16, tag="vst",
                                              bufs=2, name="vstage")
                            nc.gpsimd.tensor_mul(vstage[:], vals[:],
                                                 rbs[:, 1, :])
                            nc.gpsimd.dma_start(vT_sb[64:128, p_, sq],
                                                vstage[:])
                            nrm.pop((p_, sqb_, "rbs"))
                    return f

                op_tiles = {}

                def op_mm(sc, nb):
                    def f():
                        pso = p2ps.tile([P, 512], f32, tag="aux", bufs=2,
                                        name="pso")
                        for kc in range(2):
                            nc.tensor.matmul(
                                pso[:],
                                vT_sb[:, kc, sc * P:(sc + 1) * P],
                                Wout_sb[:, kc, nb * 512:(nb + 1) * 512],
                                start=(kc == 0), stop=(kc == 1))
                        op_tiles[(sc, nb)] = pso
                    return f

                def op_out(sc, nb, eng):
                    def f():
                        pso = op_tiles.pop((sc, nb))
                        ost = p2s.tile([P, 512], bf16, tag="ost", bufs=3,
                                       name="ost")
                        if eng == "act":
                            nc.scalar.copy(ost[:], pso[:])
                        else:
                            nc.vector.tensor_copy(ost[:], pso[:])
                        nc.gpsimd.dma_start(
                            out_d[sc * P:(sc + 1) * P,
                                  nb * 512:(nb + 1) * 512], ost[:])
                    return f

                # ---- phase 1 head: qk projection for the first head pair
                # (m=0,1), sb-major so the first chain only needs the sb=0
                # quarter of yT; m=2,3 and the v projection are filler
                # inside block 0.
                for sb in range(4):
                    for m in (0, 1):
                        proj_unit(m, sb)()

                prev_ex = None
                for bi, (p_, sqb_) in enumerate(BLOCKS):
                    sq = slice(sqb_ * SQB, (sqb_ + 1) * SQB)
                    slots = [[] for _ in range(NSC)]
                    tail = []

                    def place(st, u):
                        (slots[st] if st is not None else tail).append(u)

                    if bi == 0:
                        units = []
                        for m in (2, 3):
                            for sb in range(4):
                                units.append(proj_unit(m, sb))
                        for sc in range(NSC):
                            units.append(vproj_unit(sc))
                        for idx, u in enumerate(units):
                            slots[idx * NSC // len(units)].append(u)
                    else:
                        pp, psq = BLOCKS[bi - 1]
                        for g in range(4):
                            for sub in (0, 1):
                                place(AV_STEPS[2 * g + sub],
                                      attnv_unit(pp, psq, sub, g, prev_ex))
                        place(8, av_copies(pp, psq, 0))
                        place(10, av_copies(pp, psq, 1))
                        place(11, norm_a(bi - 1))
                    if bi >= 2:
                        place(3, norm_b1(bi - 2))
                        place(5, norm_b2(bi - 2, 0))
                        place(7, norm_b2(bi - 2, 1))
                    # out-proj tiles for sqb k: second half of the previous
                    # k early, first half of this k late (block 7: all of
                    # k=2 in the late steps)
                    ops = []
                    if bi in (4, 6):
                        k = (bi - 4) // 2
                        ops = [(k, i, OP_B[i - 4]) for i in range(4, 8)]
                    if bi in (3, 5):
                        k = (bi - 3) // 2
                        ops += [(k, i, OP_A[i]) for i in range(4)]
                    if bi == 7:
                        ops = [(2, i, OP_FULL[i]) for i in range(8)]
                    for k, i, (st_mm, st_cp) in ops:
                        sc, nb = 4 * k + i // 2, i % 2
                        place(st_mm, op_mm(sc, nb))
                        place(st_cp, op_out(sc, nb,
                                            "act" if i % 2 else "dve"))
                    if bi == 7:
                        # interleave the last block's own attn@v so only
                        # its final groups spill past the step loop
                        for g in range(3):
                            for sub in (0, 1):
                                st = 11 + 2 * g + sub
                                place(st if st < NSC else None,
                                      attnv_unit(1, 3, sub, g, None))

                    ex_t = p2e.tile([P, NSC, 2, SQB], bf16, tag="exp",
                                    bufs=2)
                    if bi == 7:
                        last_ex[0] = ex_t
                    offl = SCHR.get(bi, ())
                    for mk in range(NSC):
                        pss = p2ps.tile([P, 2, SQB], f32, tag="score",
                                        bufs=2, name="pss")
                        for sub in range(2):
                            prt = slice(sub * 64, (sub + 1) * 64)
                            nc.tensor.matmul(
                                pss[:, sub, :],
                                qkT_sb[prt, 2 * p_ + 1, mk * P:(mk + 1) * P],
                                qkT_sb[prt, 2 * p_, sq])
                        if mk in offl:
                            nc.vector.tensor_scalar(
                                ex_t[:, mk, :, :].bitcast(i16), pss[:],
                                EXP_A, EXP_B, ALU.mult, ALU.add)
                        else:
                            nc.scalar.activation(
                                ex_t[:, mk, :, :], pss[:], AF.Exp,
                                scale=0.125)
                        for u in slots[mk]:
                            u()
                    for u in tail:
                        u()
                    prev_ex = ex_t

                # epilogue: finish the last block's attn@v, normalize,
                # final out proj
                norm_b1(6)()
                attnv_unit(1, 3, 0, 3, prev_ex)()
                av_copies(1, 3, 0)()
                attnv_unit(1, 3, 1, 3, prev_ex)()
                av_copies(1, 3, 1)()
                norm_b2(6, 0)()
                norm_b2(6, 1)()
                # gen 7 takes the short tail: per-row reciprocal of the
                # denominator pair, PE-matmul partition broadcast (no DRAM
                # bounce), DVE multiply straight from PSUM.
                sq7 = slice(3 * SQB, 4 * SQB)
                den7 = nrm.pop((1, 3, "den"))
                rrow = p2s.tile([P, 2, SQB], f32, tag="rrow", bufs=1,
                                name="rrow")
                nc.vector.reciprocal(rrow[64:65, :, :], den7[64:65, :, :])
                for sub in (0, 1):
                    rps = p2ps.tile([64, SQB], f32, tag="score", bufs=2,
                                    name="rps")
                    nc.tensor.matmul(rps[:], ones_row[64:65, :],
                                     rrow[64:65, sub, :])
                    vals = nrm.pop((1, 3, sub))
                    if sub == 0:
                        nc.vector.tensor_mul(vT_sb[0:64, 1, sq7], vals[:],
                                             rps[:])
                    else:
                        vstage = p2s.tile([64, SQB], bf16, tag="vst",
                                          bufs=2, name="vstage")
                        nc.vector.tensor_mul(vstage[:], vals[:], rps[:])
                        nc.sync.dma_start(vT_sb[64:128, 1, sq7], vstage[:])
                for i in range(8):
                    sc, nb = 12 + i // 2, i % 2
                    op_mm(sc, nb)()
                    op_out(sc, nb, "act" if i % 2 else "dve")()

            p1.release()

    nc.compile()
    return nc


def _get_nc():
    if "nc" not in _CACHE:
        _CACHE["nc"] = _build_nc()
    return _CACHE["nc"]


def _host_prep(y, W_qkv, b_qkv, W_out, c):
    b = c // 4
    q = c % 4
    hs = [4 * q + i for i in range(4)]

    def Wrow(h, part):
        return W_qkv[h * 192 + part * 64: h * 192 + (part + 1) * 64]

    def brow(h, part):
        return b_qkv[h * 192 + part * 64: h * 192 + (part + 1) * 64]

    qk_rows = np.concatenate([
        Wrow(hs[0], 0), Wrow(hs[1], 0), Wrow(hs[0], 1), Wrow(hs[1], 1),
        Wrow(hs[2], 0), Wrow(hs[3], 0), Wrow(hs[2], 1), Wrow(hs[3], 1)],
        axis=0)
    bqk_flat = np.concatenate([
        brow(hs[0], 0), brow(hs[1], 0), brow(hs[0], 1), brow(hs[1], 1),
        brow(hs[2], 0), brow(hs[3], 0), brow(hs[2], 1), brow(hs[3], 1)],
        axis=0)
    import ml_dtypes

    bf = ml_dtypes.bfloat16
    WqkT = np.ascontiguousarray(qk_rows.T.astype(bf))        # [1024, 512]
    bqk = np.ascontiguousarray(bqk_flat.reshape(4, P).T)     # [128, 4]
    WvT = np.ascontiguousarray(
        np.concatenate([Wrow(h, 2) for h in hs], axis=0).T.astype(bf))
    dsl = np.concatenate([np.arange(h * 64, (h + 1) * 64) for h in hs])
    WoutT = np.ascontiguousarray(W_out[:, dsl].T.astype(bf))  # [256, 1024]
    yT = np.ascontiguousarray(y[b].T.astype(bf))             # [1024, 2048]
    return {"yT": yT, "WqkT": WqkT, "bqk": bqk, "WvT": WvT,
            "WoutT": WoutT}


def _gather(results, b_qkv, W_out, b_out):
    parts = [np.asarray(results[c]["out"], dtype=np.float32)
             for c in range(8)]
    # v-bias commutes through the output projection: fold it host-side
    bv_full = b_qkv.reshape(16, 3, 64)[:, 2, :].reshape(1024)
    bias = b_out + bv_full @ W_out.T
    return np.stack([
        parts[0] + parts[1] + parts[2] + parts[3] + bias,
        parts[4] + parts[5] + parts[6] + parts[7] + bias,
    ]).astype(np.float32)


def kernel(y, W_qkv, b_qkv, W_out, b_out):
    from concourse.bass_utils import run_bass_kernel_spmd

    y = np.ascontiguousarray(np.asarray(y, dtype=np.float32))
    W_qkv = np.ascontiguousarray(np.asarray(W_qkv, dtype=np.float32))
    b_qkv = np.ascontiguousarray(np.asarray(b_qkv, dtype=np.float32))
    W_out = np.ascontiguousarray(np.asarray(W_out, dtype=np.float32))
    b_out = np.asarray(b_out, dtype=np.float32)

    nc = _get_nc()
    in_maps = [_host_prep(y, W_qkv, b_qkv, W_out, c) for c in range(8)]
    res = run_bass_kernel_spmd(nc, in_maps, core_ids=list(range(8)))
    return _gather(res.results, b_qkv, W_out, b_out)


# revision 25
# speedup vs baseline: 1.0369x; 1.0369x over previous
"""Multi-head attention (B=2, S=2048, D=1024, H=16) on 8 TRN2 NeuronCores.

Sharding: core c handles batch b = c//4 and heads [4*(c%4), 4*(c%4)+4) —
tensor-parallel over heads x data-parallel over batch.  Each core computes a
partial output projection (its heads' contribution); the host sums the 4
partials per batch and adds b_out.

Schedule (per core): the kernel is paced so the PE never idles long enough
for the HAM clock-gate to re-throttle it to 1.2 GHz:
  - 16 "steps" per attention block, one sk-chunk each: 2 concurrent score
    matmuls (row-bands 0-63/64-127 for the head pair) into one 2-bank PSUM
    tile, then ~1 filler matmul group, then one exp covering both heads.
  - filler groups are the previous block's attn@v chains (split into 4-matmul
    groups), the v/qk projections (first block), and the output projection
    (later blocks), so PE work is dense through the ACT-paced score phase.
  - a tunable subset of exp steps runs on the DVE as a Schraudolph bit-trick
    (u16 = round(a*score + b) bitcast bf16) so the ACT engine stays strictly
    ahead of the PE.
  - softmax denominators (ones-column rows from attn@v) are transposed into
    partition-columns by DMA, inverted in one cheap batched DVE reciprocal,
    bounced through DRAM for the partition-broadcast, and multiplied into
    the bf16 vT staging the output projection consumes.
"""
import sys

sys.path.insert(0, "/opt/trn_rl_repo")

import numpy as np

B, S, D = 2, 2048, 1024
H, Hd = 16, 64
P = 128
NKC = D // P      # 8 contraction chunks for the projections
NSC = S // P      # 16 sequence chunks of 128
SQB = 512         # sq block size
NSQB = S // SQB   # 4

# Schraudolph exp on DVE: bf16 bits = EXP_A * score + EXP_B
EXP_A = 184.66246 * 0.125
EXP_B = 16249.0

# blocks processed in (sqb-major, p-interleaved) order
BLOCKS = [(0, 0), (1, 0), (0, 1), (1, 1), (0, 2), (1, 2), (0, 3), (1, 3)]
# steps whose exp runs on DVE (Schraudolph) instead of ACT, per block index
SCHR = {
    1: (1, 3, 5, 7, 9, 11, 13),
    2: (1, 3, 5, 7, 9, 11, 13),
    3: (3, 5, 7, 9, 11, 13),
    4: (3, 5, 7, 9, 11, 13),
    5: (3, 5, 7, 9, 11, 13),
    6: (3, 5, 7, 9, 11, 13),
    7: (3, 5, 7, 9, 11, 13),
}
# step placement of the previous block's attn@v 4-matmul groups, (g, sub)
# pairs in order: chains end by step 9 so the denominator/value evacuation
# and the normalize chain complete well before the next block consumes them
AV_STEPS = (1, 2, 3, 4, 5, 6, 7, 9)
# out-proj (mm_step, cast_step) placement: first half of a sqb's tiles in
# the late steps of one block, second half early in the next (vT for the
# second head pair lands ~step 6); None = after the step loop
OP_A = ((11, 13), (12, 14), (13, 15), (14, None))
OP_B = ((0, 2), (1, 3), (2, 4), (3, 5))
OP_FULL = ((8, 10), (9, 11), (10, 12), (11, 13), (12, 14), (13, 15),
           (14, None), (15, None))

_CACHE = {}


def _build_nc():
    import concourse.mybir as mybir
    import concourse.tile as tile
    from concourse import bacc

    f32 = mybir.dt.float32
    bf16 = mybir.dt.bfloat16
    i16 = mybir.dt.int16
    AF = mybir.ActivationFunctionType
    ALU = mybir.AluOpType

    nc = bacc.Bacc(None, target_bir_lowering=False, debug=False)

    yT_d = nc.dram_tensor("yT", [D, S], bf16, kind="ExternalInput")[:]
    Wqk_d = nc.dram_tensor("WqkT", [D, 512], bf16, kind="ExternalInput")[:]
    bqk_d = nc.dram_tensor("bqk", [P, 4], f32, kind="ExternalInput")[:]
    Wv_d = nc.dram_tensor("WvT", [D, 256], bf16, kind="ExternalInput")[:]
    Wout_d = nc.dram_tensor("WoutT", [256, D], bf16, kind="ExternalInput")[:]
    out_d = nc.dram_tensor("out", [S, D], bf16, kind="ExternalOutput")[:]

    with tile.TileContext(nc) as tc:
        with (
            tc.tile_pool(name="const", bufs=1) as const,
            tc.tile_pool(name="persist", bufs=1) as persist,
        ):
            qkT_sb = persist.tile([P, 4, S], bf16)
            v_sb = persist.tile([P, NSC, 4, 65], bf16)
            vT_sb = persist.tile([P, 2, S], bf16)
            den_cols = persist.tile([P, 64], f32)
            rec_cols = persist.tile([P, 64], f32)
            Wout_sb = const.tile([P, 2, D], bf16)
            bqk_sb = const.tile([P, 4], f32)
            ones_f32 = const.tile([P, 1], f32)
            ones_row = const.tile([P, 64], f32)

            p1 = tc.alloc_tile_pool(name="p1", bufs=1)
            Wqk_sb = p1.tile([P, NKC, 512], bf16)
            Wv_sb = p1.tile([P, NKC, 256], bf16)
            yT_sb = p1.tile([P, NKC, S], bf16)

            # input DMAs: per-chunk, alternating the two HWDGE queues, in
            # first-use order (Wqk + the sb=0 quarter of yT feed the first
            # projection chains) so the PE starts after ~2 MB, not 6.
            Wqkr = Wqk_d.rearrange("(kc p) e -> p kc e", p=P)
            yTr = yT_d.rearrange("(kc p) s -> p kc s", p=P)
            Wvr = Wv_d.rearrange("(kc p) e -> p kc e", p=P)
            nc.sync.dma_start(bqk_sb[:], bqk_d)
            for kc in range(NKC):
                nc.sync.dma_start(Wqk_sb[:, kc], Wqkr[:, kc])
            for kp in range(4):
                nc.scalar.dma_start(yT_sb[:, 2 * kp:2 * kp + 2, 0:512],
                                    yTr[:, 2 * kp:2 * kp + 2, 0:512])
            for sb in range(1, 4):
                for kp in range(4):
                    eng = nc.gpsimd if sb >= 2 else nc.scalar
                    eng.dma_start(
                        yT_sb[:, 2 * kp:2 * kp + 2,
                              sb * 512:(sb + 1) * 512],
                        yTr[:, 2 * kp:2 * kp + 2,
                            sb * 512:(sb + 1) * 512])
            for h in range(2):
                nc.gpsimd.dma_start(Wv_sb[:, 4 * h:4 * h + 4],
                                    Wvr[:, 4 * h:4 * h + 4])
            nc.sync.dma_start(
                Wout_sb[:], Wout_d.rearrange("(kc p) e -> p kc e", p=P))
            nc.any.memset(ones_f32[:], 1.0)
            nc.any.memset(ones_row[:], 1.0)
            nc.vector.tensor_copy(
                v_sb[:, :, :, 64:65],
                ones_f32.unsqueeze(1).unsqueeze(1).to_broadcast(
                    (P, NSC, 4, 1)))

            with (
                tc.tile_pool(name="p2e", bufs=1) as p2e,
                tc.tile_pool(name="p2s", bufs=1) as p2s,
                tc.tile_pool(name="p2ps", bufs=1, space="PSUM") as p2ps,
                tc.tile_pool(name="p2dram", bufs=2, space="DRAM") as p2dram,
            ):
                # ---- matmul unit generators (each ~one 4-8 matmul group,
                # issued between score steps as PE filler) ----
                def proj_unit(m, sb):
                    def f():
                        ps = p2ps.tile([P, 512], f32, tag="pjav", bufs=2,
                                       name="ps")
                        for kc in range(NKC):
                            nc.tensor.matmul(
                                ps[:],
                                Wqk_sb[:, kc, m * P:(m + 1) * P],
                                yT_sb[:, kc, sb * 512:(sb + 1) * 512],
                                start=(kc == 0), stop=(kc == NKC - 1))
                        nc.scalar.activation(
                            qkT_sb[:, m, sb * 512:(sb + 1) * 512], ps[:],
                            AF.Identity, bias=bqk_sb[:, m:m + 1])
                    return f

                def vproj_unit(sc):
                    def f():
                        psv = p2ps.tile([P, 256], f32, tag="aux", bufs=2,
                                        name="psv")
                        for kc in range(NKC):
                            nc.tensor.matmul(
                                psv[:],
                                yT_sb[:, kc, sc * P:(sc + 1) * P],
                                Wv_sb[:, kc, :],
                                start=(kc == 0), stop=(kc == NKC - 1))
                        nc.vector.tensor_copy(
                            v_sb[:, sc, :, 0:64],
                            psv.rearrange("p (i d) -> p i d", i=4))
                    return f

                av_tiles = {}
                nrm = {}
                last_ex = [None]

                def attnv_unit(p_, sqb_, sub, g, ex_t):
                    def f():
                        ex = ex_t if ex_t is not None else last_ex[0]
                        key = (p_, sqb_, sub)
                        if g == 0:
                            av_tiles[key] = p2ps.tile(
                                [P, SQB], f32, tag="pjav", bufs=2,
                                name="psv2")
                        psv2 = av_tiles[key]
                        i = 2 * p_ + sub
                        for mk in range(4 * g, 4 * g + 4):
                            nc.tensor.matmul(
                                psv2[0:65, :],
                                v_sb[:, mk, i, :],
                                ex[:, mk, sub, :],
                                start=(mk == 0), stop=(mk == NSC - 1))
                    return f

                def av_copies(p_, sqb_, sub):
                    """Evacuate one finished attn@v PSUM tile: den row and
                    unnormalized values to SBUF, split across ACT/DVE."""
                    def f():
                        psv2 = av_tiles.pop((p_, sqb_, sub))
                        if sub == 0:
                            nrm[(p_, sqb_, "den")] = p2s.tile(
                                [P, 2, SQB], f32, tag="den", bufs=2,
                                name="den_sb")
                        den_sb = nrm[(p_, sqb_, "den")]
                        vals = p2s.tile([64, SQB], bf16, tag="vals",
                                        bufs=4, name="vals")
                        if sub == 0:
                            nc.scalar.copy(den_sb[64:65, 0, :],
                                           psv2[64:65, :])
                            nc.vector.tensor_copy(vals[:], psv2[0:64, :])
                        else:
                            nc.vector.tensor_copy(den_sb[64:65, 1, :],
                                                  psv2[64:65, :])
                            nc.scalar.copy(vals[:], psv2[0:64, :])
                        nrm[(p_, sqb_, sub)] = vals
                    return f

                def norm_a(bi_prev):
                    """Denominator rows -> DRAM -> transposed columns
                    (DMA only, no engine time)."""
                    def f():
                        p_, sqb_ = BLOCKS[bi_prev]
                        base = bi_prev * 8
                        den_sb = nrm[(p_, sqb_, "den")]
                        den_dram = p2dram.tile([2, SQB], f32,
                                               name="den_dram", bufs=2)
                        for s in (0, 1):
                            nc.gpsimd.dma_start(den_dram[s:s + 1, :],
                                                den_sb[64:65, s, :])
                        nc.gpsimd.dma_start(
                            den_cols[:, base:base + 8].rearrange(
                                "p (s j) -> p s j", s=2),
                            den_dram.rearrange("s (j p) -> p s j", p=P))
                    return f

                def norm_b1(bi_prev):
                    """Batched reciprocal of the transposed denominators,
                    then DRAM bounce for the partition broadcast."""
                    def f():
                        p_, sqb_ = BLOCKS[bi_prev]
                        base = bi_prev * 8
                        nrm.pop((p_, sqb_, "den"))
                        nc.vector.reciprocal(rec_cols[:, base:base + 8],
                                             den_cols[:, base:base + 8])
                        rdram = p2dram.tile([2, SQB], f32, name="rdram",
                                            bufs=2)
                        nc.sync.dma_start(
                            rdram.rearrange("s (j p) -> p s j", p=P),
                            rec_cols[:, base:base + 8].rearrange(
                                "p (s j) -> p s j", s=2))
                        rbs = p2s.tile([64, 2, SQB], f32, tag="rbs",
                                       bufs=2, name="rbs")
                        for s in (0, 1):
                            nc.sync.dma_start(
                                rbs[:, s, :],
                                rdram[s:s + 1, :].to_broadcast((64, SQB)))
                        nrm[(p_, sqb_, "rbs")] = rbs
                    return f

                def norm_b2(bi_prev, sub):
                    """Normalize one head's values into vT_sb."""
                    def f():
                        p_, sqb_ = BLOCKS[bi_prev]
                        sq = slice(sqb_ * SQB, (sqb_ + 1) * SQB)
                        rbs = nrm[(p_, sqb_, "rbs")]
                        vals = nrm.pop((p_, sqb_, sub))
                        if sub == 0:
                            nc.gpsimd.tensor_mul(vT_sb[0:64, p_, sq],
                                                 vals[:], rbs[:, 0, :])
                        else:
                            vstage = p2s.tile([64, SQB], bf16, tag="vst",
                                              bufs=2, name="vstage")
                            nc.gpsimd.tensor_mul(vstage[:], vals[:],
                                                 rbs[:, 1, :])
                            nc.gpsimd.dma_start(vT_sb[64:128, p_, sq],
                                                vstage[:])
                            nrm.pop((p_, sqb_, "rbs"))
                    return f

                op_tiles = {}

                def op_mm(sc, nb):
                    def f():
                        pso = p2ps.tile([P, 512], f32, tag="aux", bufs=2,
                                        name="pso")
                        for kc in range(2):
                            nc.tensor.matmul(
                                pso[:],
                                vT_sb[:, kc, sc * P:(sc + 1) * P],
                                Wout_sb[:, kc, nb * 512:(nb + 1) * 512],
                                start=(kc == 0), stop=(kc == 1))
                        op_tiles[(sc, nb)] = pso
                    return f

                def op_out(sc, nb, eng):
                    def f():
                        pso = op_tiles.pop((sc, nb))
                        ost = p2s.tile([P, 512], bf16, tag="ost", bufs=3,
                                       name="ost")
                        if eng == "act":
                            nc.scalar.copy(ost[:], pso[:])
                        else:
                            nc.vector.tensor_copy(ost[:], pso[:])
                        nc.gpsimd.dma_start(
                            out_d[sc * P:(sc + 1) * P,
                                  nb * 512:(nb + 1) * 512], ost[:])
                    return f

                # ---- phase 1 head: qk projection for the first head pair
                # (m=0,1), sb-major so the first chain only needs the sb=0
                # quarter of yT; m=2,3 and the v projection are filler
                # inside block 0.
                for sb in range(4):
                    for m in (0, 1):
                        proj_unit(m, sb)()

                prev_ex = None
                for bi, (p_, sqb_) in enumerate(BLOCKS):
                    sq = slice(sqb_ * SQB, (sqb_ + 1) * SQB)
                    slots = [[] for _ in range(NSC)]
                    tail = []

                    def place(st, u):
                        (slots[st] if st is not None else tail).append(u)

                    if bi == 0:
                        units = []
                        for m in (2, 3):
                            for sb in range(4):
                                units.append(proj_unit(m, sb))
                        for sc in range(NSC):
                            units.append(vproj_unit(sc))
                        for idx, u in enumerate(units):
                            slots[idx * NSC // len(units)].append(u)
                    else:
                        pp, psq = BLOCKS[bi - 1]
                        for g in range(4):
                            for sub in (0, 1):
                                place(AV_STEPS[2 * g + sub],
                                      attnv_unit(pp, psq, sub, g, prev_ex))
                        place(8, av_copies(pp, psq, 0))
                        place(10, av_copies(pp, psq, 1))
                        place(11, norm_a(bi - 1))
                    if bi >= 2:
                        place(3, norm_b1(bi - 2))
                        place(5, norm_b2(bi - 2, 0))
                        place(7, norm_b2(bi - 2, 1))
                    # out-proj tiles for sqb k: second half of the previous
                    # k early, first half of this k late (block 7: all of
                    # k=2 in the late steps)
                    ops = []
                    if bi in (4, 6):
                        k = (bi - 4) // 2
                        ops = [(k, i, OP_B[i - 4]) for i in range(4, 8)]
                    if bi in (3, 5):
                        k = (bi - 3) // 2
                        ops += [(k, i, OP_A[i]) for i in range(4)]
                    if bi == 7:
                        ops = [(2, i, OP_FULL[i]) for i in range(8)]
                    for k, i, (st_mm, st_cp) in ops:
                        sc, nb = 4 * k + i // 2, i % 2
                        place(st_mm, op_mm(sc, nb))
                        place(st_cp, op_out(sc, nb,
                                            "act" if i % 2 else "dve"))
                    if bi == 7:
                        # interleave the last block's own attn@v so only
                        # its final groups spill past the step loop
                        for g in range(3):
                            for sub in (0, 1):
                                st = 11 + 2 * g + sub
                                place(st if st < NSC else None,
                                      attnv_unit(1, 3, sub, g, None))

                    ex_t = p2e.tile([P, NSC, 2, SQB], bf16, tag="exp",
                                    bufs=2)
                    if bi == 7:
                        last_ex[0] = ex_t
                    offl = SCHR.get(bi, ())
                    for mk in range(NSC):
                        pss = p2ps.tile([P, 2, SQB], f32, tag="score",
                                        bufs=2, name="pss")
                        for sub in range(2):
                            prt = slice(sub * 64, (sub + 1) * 64)
                            nc.tensor.matmul(
                                pss[:, sub, :],
                                qkT_sb[prt, 2 * p_ + 1, mk * P:(mk + 1) * P],
                                qkT_sb[prt, 2 * p_, sq])
                        if mk in offl:
                            nc.vector.tensor_scalar(
                                ex_t[:, mk, :, :].bitcast(i16), pss[:],
                                EXP_A, EXP_B, ALU.mult, ALU.add)
                        else:
                            nc.scalar.activation(
                                ex_t[:, mk, :, :], pss[:], AF.Exp,
                                scale=0.125)
                        for u in slots[mk]:
                            u()
                    for u in tail:
                        u()
                    prev_ex = ex_t

                # epilogue: finish the last block's attn@v, normalize,
                # final out proj
                # gen 7 takes the short tail: reciprocal straight off
                # the attn@v PSUM denominator rows (no den copies), started
                # the moment each sub's chain ends; PE-matmul partition
                # broadcast (no DRAM bounce); gen 6 runs its standard
                # normalize on GpSimd in parallel.
                sq7 = slice(3 * SQB, 4 * SQB)
                rrow = p2s.tile([P, 2, SQB], f32, tag="rrow", bufs=1,
                                name="rrow")
                norm_b1(6)()
                attnv_unit(1, 3, 0, 3, prev_ex)()
                psv2a = av_tiles.pop((1, 3, 0))
                valsa = p2s.tile([64, SQB], bf16, tag="vals", bufs=4,
                                 name="vals")
                nc.vector.reciprocal(rrow[64:65, 0, :], psv2a[64:65, :])
                nc.scalar.copy(valsa[:], psv2a[0:64, :])
                attnv_unit(1, 3, 1, 3, prev_ex)()
                psv2b = av_tiles.pop((1, 3, 1))
                valsb = p2s.tile([64, SQB], bf16, tag="vals", bufs=4,
                                 name="vals")
                nc.vector.reciprocal(rrow[64:65, 1, :], psv2b[64:65, :])
                nc.scalar.copy(valsb[:], psv2b[0:64, :])
                norm_b2(6, 0)()
                norm_b2(6, 1)()
                for sub, vals in ((0, valsa), (1, valsb)):
                    rps = p2ps.tile([64, SQB], f32, tag="score", bufs=2,
                                    name="rps")
                    nc.tensor.matmul(rps[:], ones_row[64:65, :],
                                     rrow[64:65, sub, :])
                    if sub == 0:
                        nc.vector.tensor_mul(vT_sb[0:64, 1, sq7], vals[:],
                                             rps[:])
                    else:
                        vstage = p2s.tile([64, SQB], bf16, tag="vst",
                                          bufs=2, name="vstage")
                        nc.vector.tensor_mul(vstage[:], vals[:], rps[:])
                        nc.sync.dma_start(vT_sb[64:128, 1, sq7], vstage[:])
                for i in range(8):
                    sc, nb = 12 + i // 2, i % 2
                    op_mm(sc, nb)()
                    op_out(sc, nb, "act" if i % 2 else "dve")()

            p1.release()

    nc.compile()
    return nc


def _get_nc():
    if "nc" not in _CACHE:
        _CACHE["nc"] = _build_nc()
    return _CACHE["nc"]


def _host_prep(y, W_qkv, b_qkv, W_out, c):
    b = c // 4
    q = c % 4
    hs = [4 * q + i for i in range(4)]

    def Wrow(h, part):
        return W_qkv[h * 192 + part * 64: h * 192 + (part + 1) * 64]

    def brow(h, part):
        return b_qkv[h * 192 + part * 64: h * 192 + (part + 1) * 64]

    qk_rows = np.concatenate([
        Wrow(hs[0], 0), Wrow(hs[1], 0), Wrow(hs[0], 1), Wrow(hs[1], 1),
        Wrow(hs[2], 0), Wrow(hs[3], 0), Wrow(hs[2], 1), Wrow(hs[3], 1)],
        axis=0)
    bqk_flat = np.concatenate([
        brow(hs[0], 0), brow(hs[1], 0), brow(hs[0], 1), brow(hs[1], 1),
        brow(hs[2], 0), brow(hs[3], 0), brow(hs[2], 1), brow(hs[3], 1)],
        axis=0)
    import ml_dtypes

    bf = ml_dtypes.bfloat16
    WqkT = np.ascontiguousarray(qk_rows.T.astype(bf))        # [1024, 512]
    bqk = np.ascontiguousarray(bqk_flat.reshape(4, P).T)     # [128, 4]
    WvT = np.ascontiguousarray(
        np.concatenate([Wrow(h, 2) for h in hs], axis=0).T.astype(bf))
    dsl = np.concatenate([np.arange(h * 64, (h + 1) * 64) for h in hs])
    WoutT = np.ascontiguousarray(W_out[:, dsl].T.astype(bf))  # [256, 1024]
    yT = np.ascontiguousarray(y[b].T.astype(bf))             # [1024, 2048]
    return {"yT": yT, "WqkT": WqkT, "bqk": bqk, "WvT": WvT,
            "WoutT": WoutT}


def _gather(results, b_qkv, W_out, b_out):
    parts = [np.asarray(results[c]["out"], dtype=np.float32)
             for c in range(8)]
    # v-bias commutes through the output projection: fold it host-side
    bv_full = b_qkv.reshape(16, 3, 64)[:, 2, :].reshape(1024)
    bias = b_out + bv_full @ W_out.T
    return np.stack([
        parts[0] + parts[1] + parts[2] + parts[3] + bias,
        parts[4] + parts[5] + parts[6] + parts[7] + bias,
    ]).astype(np.float32)


def kernel(y, W_qkv, b_qkv, W_out, b_out):
    from concourse.bass_utils import run_bass_kernel_spmd

    y = np.ascontiguousarray(np.asarray(y, dtype=np.float32))
    W_qkv = np.ascontiguousarray(np.asarray(W_qkv, dtype=np.float32))
    b_qkv = np.ascontiguousarray(np.asarray(b_qkv, dtype=np.float32))
    W_out = np.ascontiguousarray(np.asarray(W_out, dtype=np.float32))
    b_out = np.asarray(b_out, dtype=np.float32)

    nc = _get_nc()
    in_maps = [_host_prep(y, W_qkv, b_qkv, W_out, c) for c in range(8)]
    res = run_bass_kernel_spmd(nc, in_maps, core_ids=list(range(8)))
    return _gather(res.results, b_qkv, W_out, b_out)
